# revision 1
# baseline (speedup 1.0000x reference)
"""Multi-head attention (B=2, S=2048, D=1024, H=16, causal) on 8 TRN2 cores.

Sharding: data-parallel over batch x tensor-parallel over heads (Megatron).
Core c handles batch b=c//4 and heads [4g, 4g+4) with g=c%4. Each core
computes its 4 heads' Q/K/V projections, causal attention, and its partial
output projection y_partial = attn_x @ W_o[:, cols].T; the host sums the 4
partials per batch.

Everything on-chip runs in transposed (feature x seq) layout so no
transposes are needed anywhere:
  QT/KT [256, 2048] = W @ x^T,  V [s, 4*65] with a fused ones-column,
  S^T[k, q] = KT_h.T @ QT_h,    P^T = exp(S^T/8) (ACT, scale folded),
  O^T_aug [65, q] = V_aug.T @ P^T  (row 64 = softmax denominator),
  attn^T = O^T[0:64] * bcast(1/denom),  y^T = WoT.T @ attn^T.
"""

import numpy as np
import ml_dtypes

B, S, D, H = 2, 2048, 1024, 16
DK = D // H  # 64
NCORES = 8
GROUPS = 4  # cores per batch
HPC = H // GROUPS  # heads per core = 4
HD = HPC * DK  # head dims per core = 256

BF16 = ml_dtypes.bfloat16

QCHUNK = 512  # q columns processed per softmax block
NCHUNKS = S // QCHUNK  # 4
KTILE = 128  # keys per matmul tile
NKT = S // KTILE  # 16
KSUPER = 2  # k-tiles per exp batch ([128, 1024] activations)

_prog_cache = {}


# --------------------------------------------------------------------------
# walrus workaround: this compiler build allows at most 1 semaphore wait per
# instruction; move excess waits onto NoOps inserted before the instruction.
def _split_excess_waits(nc):
    import concourse.mybir as mybir

    ctr = 0
    for f in nc.m.functions:
        for bb in f.blocks:
            out = []
            changed = False
            for inst in bb.instructions:
                si = inst.sync_info
                if si is not None and si.on_wait and len(si.on_wait) > 1:
                    waits = list(si.on_wait)
                    excess, keep = waits[:-1], waits[-1:]
                    for w in excess:
                        nop = mybir.InstNoOp(name=f"waitsplit-{ctr}", ins=[], outs=[])
                        ctr += 1
                        nop.engine = inst.engine
                        nop.sync_info = mybir.SyncInfo(on_wait=[w], on_update=[])
                        out.append(nop)
                    si.on_wait = keep
                    changed = True
                out.append(inst)
            if changed:
                bb.instructions = out
    return ctr


def _build_program(causal: bool):
    import concourse.bass as bass
    import concourse.mybir as mybir
    import concourse.tile as tile

    fp32 = mybir.dt.float32
    bf16 = mybir.dt.bfloat16

    nc = bass.Bass()

    xqT = nc.dram_tensor("xqT", [D, S], bf16, kind="ExternalInput")
    xkT = nc.dram_tensor("xkT", [D, S], bf16, kind="ExternalInput")
    xvT = nc.dram_tensor("xvT", [D, S], bf16, kind="ExternalInput")
    wqT = nc.dram_tensor("wqT", [D, HD], bf16, kind="ExternalInput")
    wkT = nc.dram_tensor("wkT", [D, HD], bf16, kind="ExternalInput")
    wvT = nc.dram_tensor("wvT", [D, HD], bf16, kind="ExternalInput")
    woT = nc.dram_tensor("woT", [HD, D], bf16, kind="ExternalInput")
    yT = nc.dram_tensor("yT", [D, S], mybir.dt.float32, kind="ExternalOutput")
    maskT = None
    if not causal:
        maskT = nc.dram_tensor("maskT", [S, S], bf16, kind="ExternalInput")

    DT = D // 128  # 8 contraction tiles for the input projections

    with tile.TileContext(nc) as tc:
        with (
            tc.tile_pool(name="wpool", bufs=1) as wpool,
            tc.tile_pool(name="res", bufs=1) as res,
            tc.tile_pool(name="xin", bufs=1) as xin,
            tc.tile_pool(name="small", bufs=1) as small,
            tc.tile_pool(name="scps", bufs=2, space="PSUM") as scps,
            tc.tile_pool(name="pvps", bufs=2, space="PSUM") as pvps,
            tc.tile_pool(name="mps", bufs=2, space="PSUM") as mps,
            tc.tile_pool(name="pt", bufs=4) as ptp,
            tc.tile_pool(name="srec", bufs=2) as srec,
            tc.tile_pool(name="ostg", bufs=3) as ostg,
            tc.tile_pool(name="mload", bufs=4) as mload,
        ):
            wq_t = [wpool.tile([128, HD], bf16, tag=f"wq{k}", name=f"wq{k}") for k in range(DT)]
            wk_t = [wpool.tile([128, HD], bf16, tag=f"wk{k}", name=f"wk{k}") for k in range(DT)]
            wv_t = [wpool.tile([128, HD], bf16, tag=f"wv{k}", name=f"wv{k}") for k in range(DT)]
            wo_t = [wpool.tile([128, D], bf16, tag=f"wo{k}", name=f"wo{k}") for k in range(HD // 128)]

            # resident activations, all in (feature x seq) layout
            qt = res.tile([128, 2, S], bf16, tag="qt")
            kt = res.tile([128, 2, S], bf16, tag="kt")
            v_sb = res.tile([128, NKT, HPC, DK + 1], bf16, tag="v")
            at = res.tile([128, 2, S], bf16, tag="at")

            nc.vector.memset(v_sb[:, :, :, DK : DK + 1], 1.0)
            ones = small.tile([128, DK], bf16, tag="ones")
            nc.vector.memset(ones, 1.0)

            emasks = {}
            if causal:
                for off in (0, 128, 256, 384):
                    m = small.tile([128, QCHUNK], bf16, tag=f"emask{off}", name=f"emask{off}")
                    nc.gpsimd.memset(m, 1.0)
                    nc.gpsimd.affine_select(
                        out=m,
                        in_=m,
                        compare_op=mybir.AluOpType.is_ge,
                        fill=0.0,
                        base=-off,
                        pattern=[[1, QCHUNK]],
                        channel_multiplier=-1,
                    )
                    emasks[off] = m

            # ---- input DMAs (issue order = arrival order) ----
            xq_t, xk_t, xv_t = [], [], []
            for w_t, wdram, x_t, xdram in (
                (wq_t, wqT, xq_t, xqT),
                (wk_t, wkT, xk_t, xkT),
                (wv_t, wvT, xv_t, xvT),
            ):
                for k in range(DT):
                    nc.sync.dma_start(
                        out=w_t[k], in_=wdram[128 * k : 128 * k + 128, :]
                    )
                    xt = xin.tile([128, S], bf16, tag=f"x{id(x_t)}_{k}", name=f"x{k}")
                    nc.sync.dma_start(
                        out=xt, in_=xdram[128 * k : 128 * k + 128, :]
                    )
                    x_t.append(xt)
            for k in range(HD // 128):
                nc.sync.dma_start(out=wo_t[k], in_=woT[128 * k : 128 * k + 128, :])

            # ---- projection-group emitters ----
            def proj_group(w_t, x_t, dst, n, m):
                ps = mps.tile([128, QCHUNK], fp32, tag="misc", name=f"pj{n}_{m}")
                for k in range(DT):
                    nc.tensor.matmul(
                        ps[:, :],
                        lhsT=w_t[k][:, 128 * m : 128 * m + 128],
                        rhs=x_t[k][:, QCHUNK * n : QCHUNK * (n + 1)],
                        start=(k == 0),
                        stop=(k == DT - 1),
                    )
                if (n + m) % 2 == 0:
                    nc.scalar.copy(
                        out=dst[:, m, QCHUNK * n : QCHUNK * (n + 1)],
                        in_=ps[:, :],
                    )
                else:
                    nc.vector.tensor_copy(
                        dst[:, m, QCHUNK * n : QCHUNK * (n + 1)], ps[:, :]
                    )

            def v_group(j):
                ps = mps.tile([128, HD], fp32, tag="misc", name=f"vj{j}")
                for k in range(DT):
                    nc.tensor.matmul(
                        ps[:, :],
                        lhsT=xv_t[k][:, 128 * j : 128 * j + 128],
                        rhs=wv_t[k][:, :],
                        start=(k == 0),
                        stop=(k == DT - 1),
                    )
                if j % 2 == 0:
                    nc.scalar.copy(
                        out=v_sb[:, j, :, 0:DK],
                        in_=ps.rearrange("p (h d) -> p h d", h=HPC),
                    )
                else:
                    nc.vector.tensor_copy(
                        v_sb[:, j, :, 0:DK],
                        ps.rearrange("p (h d) -> p h d", h=HPC),
                    )

            # chunk 0 prerequisites up front
            for m in range(2):
                proj_group(wq_t, xq_t, qt, 0, m)
            for m in range(2):
                proj_group(wk_t, xk_t, kt, 0, m)

            # later QT/KT chunks, V tiles, and output projections stream into
            # the attention loop slots: queues[c] pops during chunk c.
            from collections import deque

            queues = [deque() for _ in range(NCHUNKS)]
            for j in range(4):
                queues[0].append(lambda j=j: v_group(j))
            for n in range(1, NCHUNKS):
                for m in range(2):
                    queues[n - 1].append(
                        lambda n=n, m=m: proj_group(wq_t, xq_t, qt, n, m)
                    )
                for m in range(2):
                    queues[n - 1].append(
                        lambda n=n, m=m: proj_group(wk_t, xk_t, kt, n, m)
                    )
                for j in range(4 * n, 4 * n + 4):
                    queues[n - 1].append(lambda j=j: v_group(j))

            # ---- fused attention + streamed projections + output proj ----
            for c in range(NCHUNKS):
                qs = slice(QCHUNK * c, QCHUNK * (c + 1))
                nkt_c = 4 * (c + 1) if causal else NKT
                nsup = (nkt_c + KSUPER - 1) // KSUPER
                nslots = 2 * nsup
                pending = queues[c]
                per_slot = (len(pending) + nslots - 1) // max(1, nslots)

                osbs = {}
                for pair in range(2):
                    hs = (2 * pair, 2 * pair + 1)
                    pvs = {
                        h: pvps.tile(
                            [DK + 1, QCHUNK], fp32, tag="pv", name=f"pv{c}_{h}"
                        )
                        for h in hs
                    }
                    for s_i in range(nsup):
                        jt = [
                            KSUPER * s_i + j2
                            for j2 in range(KSUPER)
                            if KSUPER * s_i + j2 < nkt_c
                        ]
                        scs = {
                            h: scps.tile(
                                [128, KSUPER * QCHUNK], fp32, tag="sc",
                                name=f"sc{c}_{s_i}_{h}",
                            )
                            for h in hs
                        }
                        # adjacent QKs alternate PE row groups (rows 0-63 for
                        # even heads, 64-127 for odd) -> run concurrently
                        for j2, j in enumerate(jt):
                            for h in hs:
                                mh = h // 2
                                ph = 64 * (h % 2)
                                nc.tensor.matmul(
                                    scs[h][:, QCHUNK * j2 : QCHUNK * (j2 + 1)],
                                    lhsT=kt[ph : ph + DK, mh, 128 * j : 128 * j + 128],
                                    rhs=qt[ph : ph + DK, mh, qs],
                                    start=True,
                                    stop=True,
                                )
                        pts = {}
                        for h in hs:
                            pt = ptp.tile(
                                [128, KSUPER * QCHUNK], bf16, tag="pt",
                                name=f"pt{c}_{s_i}_{h}",
                            )
                            pts[h] = pt
                            nsc = QCHUNK * len(jt)
                            nc.scalar.activation(
                                out=pt[:, 0:nsc],
                                in_=scs[h][:, 0:nsc],
                                func=mybir.ActivationFunctionType.Exp,
                                scale=1.0 / np.sqrt(np.float32(DK)),
                            )
                        emitted = 0
                        while pending and emitted < per_slot:
                            pending.popleft()()
                            emitted += 1
                        for h in hs:
                            pt = pts[h]
                            for j2, j in enumerate(jt):
                                pslice = pt[:, QCHUNK * j2 : QCHUNK * (j2 + 1)]
                                if causal:
                                    off = 128 * j - QCHUNK * c
                                    if off >= 0:
                                        nc.gpsimd.tensor_mul(
                                            out=pslice, in0=pslice, in1=emasks[off]
                                        )
                                else:
                                    mt = mload.tile(
                                        [128, QCHUNK], bf16, tag="mt",
                                        name=f"mt{c}_{s_i}_{h}_{j2}",
                                    )
                                    nc.sync.dma_start(
                                        out=mt,
                                        in_=maskT[128 * j : 128 * j + 128, qs],
                                    )
                                    nc.vector.tensor_mul(
                                        out=pslice, in0=pslice, in1=mt
                                    )
                                nc.tensor.matmul(
                                    pvs[h][0 : DK + 1, :],
                                    lhsT=v_sb[:, j, h, :],
                                    rhs=pslice,
                                    start=(j == 0),
                                    stop=(j == nkt_c - 1),
                                )
                    # drain this pair's PV banks to SBUF
                    for h in hs:
                        if not causal:
                            nc.scalar.add(
                                out=pvs[h][DK : DK + 1, :],
                                in_=pvs[h][DK : DK + 1, :],
                                add=1e-30,
                            )
                        o = srec.tile(
                            [DK + 1, QCHUNK], fp32, tag="osb", bufs=9,
                            name=f"osb{c}_{h}",
                        )
                        nc.vector.tensor_copy(o[:, :], pvs[h][0 : DK + 1, :])
                        osbs[h] = o

                # normalize all 4 heads; streamed into the next chunk's
                # slots so the bcast matmuls never head-of-line-block the PE.
                def normalize(c, osbs, qs=qs):
                    recf = srec.tile(
                        [128, QCHUNK], fp32, tag="recf", name=f"recf{c}"
                    )
                    recb = srec.tile(
                        [128, QCHUNK], bf16, tag="recb", name=f"recb{c}"
                    )
                    for h in range(HPC):
                        nc.vector.tensor_copy(
                            recf[32 * h : 32 * h + 1, :],
                            osbs[h][DK : DK + 1, :],
                        )
                    with nc.allow_low_precision(reason="softmax denom bf16"):
                        nc.vector.reciprocal(
                            out=recb[0:97, :], in_=recf[0:97, :]
                        )
                    for h in range(HPC):
                        mh = h // 2
                        ph = 64 * (h % 2)
                        bc = mps.tile(
                            [DK, QCHUNK], fp32, tag="misc", name=f"bc{c}_{h}"
                        )
                        nc.tensor.matmul(
                            bc[:, :],
                            lhsT=ones[32 * h : 32 * h + 1, :],
                            rhs=recb[32 * h : 32 * h + 1, :],
                            start=True,
                            stop=True,
                            tile_position=(32 * h, 0),
                        )
                        nc.vector.tensor_mul(
                            out=at[ph : ph + DK, mh, qs],
                            in0=osbs[h][0:DK, :],
                            in1=bc[:, :],
                        )

                if c + 1 < NCHUNKS:
                    queues[c + 1].append(lambda c=c, osbs=osbs: normalize(c, osbs))
                else:
                    normalize(c, osbs)

                # output projection for this chunk streams into the next
                # chunk's slots (chunk 3's runs right here).
                def op_group(c, mo, qs=qs):
                    ps = mps.tile(
                        [128, QCHUNK], fp32, tag="misc", name=f"op{c}_{mo}"
                    )
                    for k2 in range(HD // 128):
                        nc.tensor.matmul(
                            ps[:, :],
                            lhsT=wo_t[k2][:, 128 * mo : 128 * mo + 128],
                            rhs=at[:, k2, qs],
                            start=(k2 == 0),
                            stop=(k2 == HD // 128 - 1),
                        )
                    stg = ostg.tile(
                        [128, QCHUNK], fp32, tag="stg", name=f"stg{c}_{mo}"
                    )
                    if mo % 2 == 0:
                        nc.scalar.copy(out=stg[:, :], in_=ps[:, :])
                    else:
                        nc.vector.tensor_copy(stg[:, :], ps[:, :])
                    nc.sync.dma_start(
                        out=yT[128 * mo : 128 * mo + 128, qs],
                        in_=stg[:, :],
                    )

                for mo in range(D // 128):
                    if c + 1 < NCHUNKS:
                        queues[c + 1].append(lambda c=c, mo=mo: op_group(c, mo))
                    else:
                        op_group(c, mo)

    _split_excess_waits(nc)
    return nc


def kernel(query, key, value, mask, W_q, W_k, W_v, W_o):
    from concourse.bass_utils import run_bass_kernel_spmd

    query = np.asarray(query)
    key = np.asarray(key)
    value = np.asarray(value)
    mask = np.asarray(mask)
    W_q = np.asarray(W_q)
    W_k = np.asarray(W_k)
    W_v = np.asarray(W_v)
    W_o = np.asarray(W_o)

    m2 = mask.reshape(mask.shape[-2], mask.shape[-1])
    causal = bool(
        np.array_equal(m2 != 0, np.tril(np.ones((S, S), dtype=bool)))
    )

    if causal not in _prog_cache:
        _prog_cache[causal] = _build_program(causal)
    nc = _prog_cache[causal]

    # host-side shard prep (bf16, transposed)
    xT = {}
    for b in range(B):
        xT[("q", b)] = np.ascontiguousarray(query[b].T).astype(BF16)
        xT[("k", b)] = np.ascontiguousarray(key[b].T).astype(BF16)
        xT[("v", b)] = np.ascontiguousarray(value[b].T).astype(BF16)
    maskT_np = None
    if not causal:
        maskT_np = np.ascontiguousarray((m2 != 0).T).astype(BF16)

    in_maps = []
    for c in range(NCORES):
        b, g = divmod(c, GROUPS)
        rows = slice(HD * g, HD * (g + 1))
        im = {
            "xqT": xT[("q", b)],
            "xkT": xT[("k", b)],
            "xvT": xT[("v", b)],
            "wqT": np.ascontiguousarray(W_q[rows, :].T).astype(BF16),
            "wkT": np.ascontiguousarray(W_k[rows, :].T).astype(BF16),
            "wvT": np.ascontiguousarray(W_v[rows, :].T).astype(BF16),
            "woT": np.ascontiguousarray(W_o[:, rows].T).astype(BF16),
        }
        if not causal:
            im["maskT"] = maskT_np
        in_maps.append(im)

    res = run_bass_kernel_spmd(nc, in_maps, core_ids=list(range(NCORES)))

    out = np.zeros((B, S, D), dtype=np.float32)
    for c in range(NCORES):
        b = c // GROUPS
        out[b] += res.results[c]["yT"].T
    return out



# revision 8
# speedup vs baseline: 1.1043x; 1.1043x over previous
"""Multi-head attention (B=2, S=2048, D=1024, H=16, causal) on 8 TRN2 cores.

Sharding: data-parallel over batch x tensor-parallel over heads (Megatron).
Core c handles batch b=c//4 and heads [4g, 4g+4) with g=c%4. Each core
computes its 4 heads' Q/K/V projections, causal attention, and its partial
output projection y_partial = attn_x @ W_o[:, cols].T; the host sums the 4
partials per batch.

Everything on-chip runs in transposed (feature x seq) layout so no
transposes are needed anywhere:
  QT/KT [256, 2048] = W @ x^T,  V [s, 4*65] with a fused ones-column,
  S^T[k, q] = KT_h.T @ QT_h,    P^T = exp(S^T/8) (ACT, scale folded),
  O^T_aug [65, q] = V_aug.T @ P^T  (row 64 = softmax denominator),
  attn^T = O^T[0:64] * bcast(1/denom),  y^T = WoT.T @ attn^T.

v2 scheduling (vs the first working version):
  - input DMAs are issued in 512-column chunks ordered so the first QK
    projection only waits on ~3MB instead of ~9MB;
  - ~20 zero-valued warmup matmuls (accumulating 0 into the first proj
    psum) keep the PE HAM clock warm through the DMA prologue;
  - all PSUM->SBUF drains go to DVE; ACT does exp exclusively (its ~92us
    of exp is the second-busiest engine after PE);
  - output-projection groups are deferred into the last (exp-heaviest)
    chunk so the PE always has independent work while ACT catches up;
  - the softmax reciprocal broadcast uses a tiny [4,128] 0/1 matrix E so
    one matmul covers a head pair;
  - yT is stored bf16 (halves output DMA; host sums partials in fp32).
"""

import numpy as np
import ml_dtypes

B, S, D, H = 2, 2048, 1024, 16
DK = D // H  # 64
NCORES = 8
GROUPS = 4  # cores per batch
HPC = H // GROUPS  # heads per core = 4
HD = HPC * DK  # head dims per core = 256

BF16 = ml_dtypes.bfloat16

QCHUNK = 512  # q columns processed per softmax block
NCHUNKS = S // QCHUNK  # 4
KTILE = 128  # keys per matmul tile
NKT = S // KTILE  # 16
KSUPER = 2  # k-tiles per exp batch ([128, 1024] activations)
NWARM = 12  # zero matmuls to warm the PE clock during the DMA prologue

_prog_cache = {}


# --------------------------------------------------------------------------
# walrus workaround: this compiler build allows at most 1 semaphore wait per
# instruction; move excess waits onto NoOps inserted before the instruction.
def _split_excess_waits(nc):
    import concourse.mybir as mybir

    ctr = 0
    for f in nc.m.functions:
        for bb in f.blocks:
            out = []
            changed = False
            for inst in bb.instructions:
                si = inst.sync_info
                if si is not None and si.on_wait and len(si.on_wait) > 1:
                    waits = list(si.on_wait)
                    excess, keep = waits[:-1], waits[-1:]
                    for w in excess:
                        nop = mybir.InstNoOp(name=f"waitsplit-{ctr}", ins=[], outs=[])
                        ctr += 1
                        nop.engine = inst.engine
                        nop.sync_info = mybir.SyncInfo(on_wait=[w], on_update=[])
                        out.append(nop)
                    si.on_wait = keep
                    changed = True
                out.append(inst)
            if changed:
                bb.instructions = out
    return ctr


def _build_program(causal: bool):
    import concourse.bass as bass
    import concourse.mybir as mybir
    import concourse.tile as tile

    fp32 = mybir.dt.float32
    bf16 = mybir.dt.bfloat16

    nc = bass.Bass()

    xqT = nc.dram_tensor("xqT", [D, S], bf16, kind="ExternalInput")
    xkT = nc.dram_tensor("xkT", [D, S], bf16, kind="ExternalInput")
    xvT = nc.dram_tensor("xvT", [D, S], bf16, kind="ExternalInput")
    wqT = nc.dram_tensor("wqT", [D, HD], bf16, kind="ExternalInput")
    wkT = nc.dram_tensor("wkT", [D, HD], bf16, kind="ExternalInput")
    wvT = nc.dram_tensor("wvT", [D, HD], bf16, kind="ExternalInput")
    woT = nc.dram_tensor("woT", [HD, D], bf16, kind="ExternalInput")
    yT = nc.dram_tensor("yT", [D, S], bf16, kind="ExternalOutput")
    maskT = None
    if not causal:
        maskT = nc.dram_tensor("maskT", [S, S], bf16, kind="ExternalInput")

    DT = D // 128  # 8 contraction tiles for the input projections

    with tile.TileContext(nc) as tc:
        with (
            tc.tile_pool(name="wpool", bufs=1) as wpool,
            tc.tile_pool(name="res", bufs=1) as res,
            tc.tile_pool(name="xin", bufs=1) as xin,
            tc.tile_pool(name="small", bufs=1) as small,
            tc.tile_pool(name="scps", bufs=2, space="PSUM") as scps,
            tc.tile_pool(name="pvps", bufs=2, space="PSUM") as pvps,
            tc.tile_pool(name="mps", bufs=2, space="PSUM") as mps,
            tc.tile_pool(name="pt", bufs=4) as ptp,
            tc.tile_pool(name="srec", bufs=2) as srec,
            tc.tile_pool(name="ostg", bufs=3) as ostg,
            tc.tile_pool(name="mload", bufs=4) as mload,
        ):
            wq_t = [wpool.tile([128, HD], bf16, tag=f"wq{k}", name=f"wq{k}") for k in range(DT)]
            wk_t = [wpool.tile([128, HD], bf16, tag=f"wk{k}", name=f"wk{k}") for k in range(DT)]
            wv_t = [wpool.tile([128, HD], bf16, tag=f"wv{k}", name=f"wv{k}") for k in range(DT)]
            wo_t = [wpool.tile([128, D], bf16, tag=f"wo{k}", name=f"wo{k}") for k in range(HD // 128)]

            # resident activations, all in (feature x seq) layout
            qt = res.tile([128, 2, S], bf16, tag="qt")
            kt = res.tile([128, 2, S], bf16, tag="kt")
            v_sb = res.tile([128, NKT, HPC, DK + 1], bf16, tag="v")
            at = res.tile([128, 2, S], bf16, tag="at")

            nc.vector.memset(v_sb[:, :, :, DK : DK + 1], 1.0)

            # E: bcast matrix; E[32*(2p+i), p, 64i:64i+64] = 1 selects head
            # (2p+i)'s reciprocal row into output partitions 64i..64i+64 for
            # pair p. Rows sit at 32-multiples (partition-base alignment).
            ebc = small.tile([97, 2, 128], bf16, tag="ebc")
            nc.vector.memset(ebc, 0.0)
            for p in range(2):
                for i in range(2):
                    h = 2 * p + i
                    nc.vector.memset(ebc[32 * h : 32 * h + 1, p, 64 * i : 64 * i + 64], 1.0)

            # warmup fodder: zeros, so warmup matmuls accumulate exact 0s.
            wub = small.tile([128, 648], bf16, tag="wub")
            nc.gpsimd.memset(wub, 0.0)
            # preload the exp activation table during the DMA prologue
            nc.scalar.activation(
                out=wub[:, 640:648],
                in_=wub[:, 640:648],
                func=mybir.ActivationFunctionType.Exp,
                scale=1.0,
            )

            emasks = {}
            if causal:
                for off in (0, 128, 256, 384):
                    m = small.tile([128, QCHUNK], bf16, tag=f"emask{off}", name=f"emask{off}")
                    nc.gpsimd.memset(m, 1.0)
                    nc.gpsimd.affine_select(
                        out=m,
                        in_=m,
                        compare_op=mybir.AluOpType.is_ge,
                        fill=0.0,
                        base=-off,
                        pattern=[[1, QCHUNK]],
                        channel_multiplier=-1,
                    )
                    emasks[off] = m

            # ---- input DMAs, chunk-granular, issue order = arrival order ----
            xq_t = [xin.tile([128, S], bf16, tag=f"xq{k}", name=f"xq{k}") for k in range(DT)]
            xk_t = [xin.tile([128, S], bf16, tag=f"xk{k}", name=f"xk{k}") for k in range(DT)]
            xv_t = [xin.tile([128, S], bf16, tag=f"xv{k}", name=f"xv{k}") for k in range(DT)]

            def dma_w(w_t, wdram):
                for k in range(DT):
                    nc.sync.dma_start(out=w_t[k], in_=wdram[128 * k : 128 * k + 128, :])

            def dma_x_chunk(x_t, xdram, c):
                cs = slice(QCHUNK * c, QCHUNK * (c + 1))
                for k in range(DT):
                    nc.sync.dma_start(
                        out=x_t[k][:, cs], in_=xdram[128 * k : 128 * k + 128, cs]
                    )

            dma_w(wq_t, wqT)
            dma_x_chunk(xq_t, xqT, 0)
            dma_w(wk_t, wkT)
            dma_x_chunk(xk_t, xkT, 0)
            dma_w(wv_t, wvT)
            dma_x_chunk(xv_t, xvT, 0)
            dma_x_chunk(xq_t, xqT, 1)
            dma_x_chunk(xk_t, xkT, 1)
            dma_x_chunk(xv_t, xvT, 1)
            for k in range(HD // 128):
                nc.sync.dma_start(out=wo_t[k], in_=woT[128 * k : 128 * k + 128, :])
            for c in range(2, NCHUNKS):
                dma_x_chunk(xq_t, xqT, c)
                dma_x_chunk(xk_t, xkT, c)
                dma_x_chunk(xv_t, xvT, c)

            # ---- projection-group emitters ----
            def proj_group(w_t, x_t, dst, n, m, warmup=0):
                ps = mps.tile([128, QCHUNK], fp32, tag="misc", name=f"pj{id(w_t)}_{n}_{m}")
                for i in range(warmup):
                    nc.tensor.matmul(
                        ps[:, :],
                        lhsT=wub[:, 0:128],
                        rhs=wub[:, 128 : 128 + QCHUNK],
                        start=(i == 0),
                        stop=False,
                    )
                for k in range(DT):
                    nc.tensor.matmul(
                        ps[:, :],
                        lhsT=w_t[k][:, 128 * m : 128 * m + 128],
                        rhs=x_t[k][:, QCHUNK * n : QCHUNK * (n + 1)],
                        start=(warmup == 0 and k == 0),
                        stop=(k == DT - 1),
                    )
                nc.vector.tensor_copy(
                    dst[:, m, QCHUNK * n : QCHUNK * (n + 1)], ps[:, :]
                )

            def v_group(j):
                ps = mps.tile([128, HD], fp32, tag="misc", name=f"vj{j}")
                for k in range(DT):
                    nc.tensor.matmul(
                        ps[:, :],
                        lhsT=xv_t[k][:, 128 * j : 128 * j + 128],
                        rhs=wv_t[k][:, :],
                        start=(k == 0),
                        stop=(k == DT - 1),
                    )
                nc.vector.tensor_copy(
                    v_sb[:, j, :, 0:DK],
                    ps.rearrange("p (h d) -> p h d", h=HPC),
                )

            # chunk-0 prerequisites up front (warmup rides the first group)
            proj_group(wq_t, xq_t, qt, 0, 0, warmup=NWARM)
            proj_group(wq_t, xq_t, qt, 0, 1)
            proj_group(wk_t, xk_t, kt, 0, 0)
            proj_group(wk_t, xk_t, kt, 0, 1)
            for j in range(4 if causal else NKT):
                v_group(j)

            # ---- deferred-work schedule ----
            # Flat slot list: one slot per (chunk, pair, s_i). Pops run after
            # the slot's exp is issued, giving ACT room while the PE works.
            nkt_of = [4 * (c + 1) if causal else NKT for c in range(NCHUNKS)]
            nsup_of = [(n + KSUPER - 1) // KSUPER for n in nkt_of]
            slot_base = {}
            total_slots = 0
            for c in range(NCHUNKS):
                for p in range(2):
                    slot_base[(c, p)] = total_slots
                    total_slots += nsup_of[c]
            pops = [[] for _ in range(total_slots)]

            def sched(c, p, s_i, fn):
                pops[slot_base[(c, p)] + s_i].append(fn)

            # projections/v for later chunks, spread just-in-time
            if causal:
                # chunk 1 deps during chunk 0 (4 slots)
                sched(0, 0, 0, lambda: proj_group(wq_t, xq_t, qt, 1, 0))
                sched(0, 0, 1, lambda: proj_group(wq_t, xq_t, qt, 1, 1))
                sched(0, 1, 0, lambda: proj_group(wk_t, xk_t, kt, 1, 0))
                sched(0, 1, 0, lambda: v_group(4))
                sched(0, 1, 1, lambda: proj_group(wk_t, xk_t, kt, 1, 1))
                sched(0, 1, 1, lambda: v_group(5))
                sched(0, 1, 1, lambda: v_group(6))
                sched(0, 1, 1, lambda: v_group(7))
                # chunk 2 deps during chunk 1 (8 slots)
                sched(1, 0, 1, lambda: proj_group(wq_t, xq_t, qt, 2, 0))
                sched(1, 0, 2, lambda: proj_group(wq_t, xq_t, qt, 2, 1))
                sched(1, 0, 3, lambda: proj_group(wk_t, xk_t, kt, 2, 0))
                sched(1, 1, 0, lambda: proj_group(wk_t, xk_t, kt, 2, 1))
                sched(1, 1, 1, lambda: v_group(8))
                sched(1, 1, 1, lambda: v_group(9))
                sched(1, 1, 2, lambda: v_group(10))
                sched(1, 1, 2, lambda: v_group(11))
                # chunk 3 deps during chunk 2 (12 slots)
                sched(2, 0, 1, lambda: proj_group(wq_t, xq_t, qt, 3, 0))
                sched(2, 0, 2, lambda: proj_group(wq_t, xq_t, qt, 3, 1))
                sched(2, 0, 3, lambda: proj_group(wk_t, xk_t, kt, 3, 0))
                sched(2, 0, 4, lambda: proj_group(wk_t, xk_t, kt, 3, 1))
                sched(2, 0, 5, lambda: v_group(12))
                sched(2, 1, 0, lambda: v_group(13))
                sched(2, 1, 1, lambda: v_group(14))
                sched(2, 1, 1, lambda: v_group(15))
            else:
                for n in range(1, NCHUNKS):
                    cprev = n - 1
                    ns = nsup_of[cprev]
                    sched(cprev, 0, 0 % ns, lambda n=n: proj_group(wq_t, xq_t, qt, n, 0))
                    sched(cprev, 0, 1 % ns, lambda n=n: proj_group(wq_t, xq_t, qt, n, 1))
                    sched(cprev, 1, 0 % ns, lambda n=n: proj_group(wk_t, xk_t, kt, n, 0))
                    sched(cprev, 1, 1 % ns, lambda n=n: proj_group(wk_t, xk_t, kt, n, 1))

            # normalize + output projection machinery ------------------------
            osbs = {}  # (c, p) -> osb_pair tile
            recfs = {}  # c -> [4, QCHUNK] fp32 denominators

            def normalize(c):
                qs = slice(QCHUNK * c, QCHUNK * (c + 1))
                recf = recfs[c]
                recb = srec.tile([97, QCHUNK], bf16, tag="recb", name=f"recb{c}")
                with nc.allow_low_precision(reason="softmax denom bf16"):
                    nc.vector.reciprocal(out=recb[0:97, :], in_=recf[0:97, :])
                for p in range(2):
                    bc = mps.tile([128, QCHUNK], fp32, tag="misc", name=f"bc{c}_{p}")
                    nc.tensor.matmul(
                        bc[:, :],
                        lhsT=ebc[0:97, p, :],
                        rhs=recb[0:97, :],
                        start=True,
                        stop=True,
                    )
                    nc.vector.tensor_mul(
                        out=at[:, p, qs],
                        in0=osbs[(c, p)][:, :],
                        in1=bc[:, :],
                    )

            def op_group(c, mo):
                qs = slice(QCHUNK * c, QCHUNK * (c + 1))
                ps = mps.tile([128, QCHUNK], fp32, tag="misc", name=f"op{c}_{mo}")
                for k2 in range(HD // 128):
                    nc.tensor.matmul(
                        ps[:, :],
                        lhsT=wo_t[k2][:, 128 * mo : 128 * mo + 128],
                        rhs=at[:, k2, qs],
                        start=(k2 == 0),
                        stop=(k2 == HD // 128 - 1),
                    )
                stg = ostg.tile([128, QCHUNK], bf16, tag="stg", name=f"stg{c}_{mo}")
                nc.vector.tensor_copy(stg[:, :], ps[:, :])
                nc.sync.dma_start(out=yT[128 * mo : 128 * mo + 128, qs], in_=stg[:, :])

            # defer normalize + op groups into the exp-heavy later chunks
            if causal:
                sched(1, 0, 0, lambda: normalize(0))
                sched(2, 0, 0, lambda: normalize(1))
                sched(3, 0, 0, lambda: normalize(2))
                for mo in range(D // 128):  # op(c0) across c3 pair 0
                    sched(3, 0, mo, lambda mo=mo: op_group(0, mo))
                for idx in range(2 * (D // 128)):  # op(c1), op(c2) across c3 pair 1
                    c_src = 1 + idx // (D // 128)
                    mo = idx % (D // 128)
                    sched(3, 1, idx // 2, lambda c_src=c_src, mo=mo: op_group(c_src, mo))
            else:
                for c in range(NCHUNKS - 1):
                    ns = nsup_of[c + 1]
                    sched(c + 1, 0, 0, lambda c=c: normalize(c))
                    for mo in range(D // 128):
                        sched(c + 1, (mo % 2), (1 + mo // 2) % ns, lambda c=c, mo=mo: op_group(c, mo))

            # ---- fused attention + streamed projections ----
            for c in range(NCHUNKS):
                qs = slice(QCHUNK * c, QCHUNK * (c + 1))
                nkt_c = nkt_of[c]
                nsup = nsup_of[c]
                # denominator rows land at partitions {0,32,64,96}; the other
                # rows are set to 1.0 so the [0:97] reciprocal stays finite
                # (ebc has zeros there, so they contribute nothing).
                recf = srec.tile([97, QCHUNK], fp32, tag="recf", name=f"recf{c}")
                recfs[c] = recf
                nc.vector.memset(recf, 1.0)

                for pair in range(2):
                    hs = (2 * pair, 2 * pair + 1)
                    pvs = {
                        h: pvps.tile(
                            [DK + 1, QCHUNK], fp32, tag="pv", name=f"pv{c}_{h}"
                        )
                        for h in hs
                    }
                    for s_i in range(nsup):
                        jt = [
                            KSUPER * s_i + j2
                            for j2 in range(KSUPER)
                            if KSUPER * s_i + j2 < nkt_c
                        ]
                        scs = {
                            h: scps.tile(
                                [128, KSUPER * QCHUNK], fp32, tag="sc",
                                name=f"sc{c}_{s_i}_{h}",
                            )
                            for h in hs
                        }
                        # adjacent QKs alternate PE row groups (rows 0-63 for
                        # even heads, 64-127 for odd) -> run concurrently
                        for j2, j in enumerate(jt):
                            for h in hs:
                                mh = h // 2
                                ph = 64 * (h % 2)
                                nc.tensor.matmul(
                                    scs[h][:, QCHUNK * j2 : QCHUNK * (j2 + 1)],
                                    lhsT=kt[ph : ph + DK, mh, 128 * j : 128 * j + 128],
                                    rhs=qt[ph : ph + DK, mh, qs],
                                    start=True,
                                    stop=True,
                                )
                        pts = {}
                        for h in hs:
                            pt = ptp.tile(
                                [128, KSUPER * QCHUNK], bf16, tag="pt",
                                name=f"pt{c}_{s_i}_{h}",
                            )
                            pts[h] = pt
                            nsc = QCHUNK * len(jt)
                            nc.scalar.activation(
                                out=pt[:, 0:nsc],
                                in_=scs[h][:, 0:nsc],
                                func=mybir.ActivationFunctionType.Exp,
                                scale=1.0 / np.sqrt(np.float32(DK)),
                            )
                        for fn in pops[slot_base[(c, pair)] + s_i]:
                            fn()
                        for h in hs:
                            pt = pts[h]
                            for j2, j in enumerate(jt):
                                pslice = pt[:, QCHUNK * j2 : QCHUNK * (j2 + 1)]
                                if causal:
                                    off = 128 * j - QCHUNK * c
                                    if off >= 0:
                                        nc.gpsimd.tensor_mul(
                                            out=pslice, in0=pslice, in1=emasks[off]
                                        )
                                else:
                                    mt = mload.tile(
                                        [128, QCHUNK], bf16, tag="mt",
                                        name=f"mt{c}_{s_i}_{h}_{j2}",
                                    )
                                    nc.sync.dma_start(
                                        out=mt,
                                        in_=maskT[128 * j : 128 * j + 128, qs],
                                    )
                                    nc.vector.tensor_mul(
                                        out=pslice, in0=pslice, in1=mt
                                    )
                                nc.tensor.matmul(
                                    pvs[h][0 : DK + 1, :],
                                    lhsT=v_sb[:, j, h, :],
                                    rhs=pslice,
                                    start=(j == 0),
                                    stop=(j == nkt_c - 1),
                                )
                    # drain this pair's PV banks: attention rows into a packed
                    # [128, QCHUNK] pair tile, denominator rows into recf.
                    o = srec.tile(
                        [128, QCHUNK], fp32, tag="osb", bufs=5,
                        name=f"osb{c}_{pair}",
                    )
                    osbs[(c, pair)] = o
                    for i, h in enumerate(hs):
                        if not causal:
                            nc.scalar.add(
                                out=pvs[h][DK : DK + 1, :],
                                in_=pvs[h][DK : DK + 1, :],
                                add=1e-30,
                            )
                        nc.vector.tensor_copy(
                            o[64 * i : 64 * i + 64, :], pvs[h][0:DK, :]
                        )
                        nc.vector.tensor_copy(
                            recf[32 * h : 32 * h + 1, :], pvs[h][DK : DK + 1, :]
                        )

            # tail: last chunk's normalize + output projection
            normalize(NCHUNKS - 1)
            for mo in range(D // 128):
                op_group(NCHUNKS - 1, mo)

    _split_excess_waits(nc)
    return nc


def kernel(query, key, value, mask, W_q, W_k, W_v, W_o):
    from concourse.bass_utils import run_bass_kernel_spmd

    query = np.asarray(query)
    key = np.asarray(key)
    value = np.asarray(value)
    mask = np.asarray(mask)
    W_q = np.asarray(W_q)
    W_k = np.asarray(W_k)
    W_v = np.asarray(W_v)
    W_o = np.asarray(W_o)

    m2 = mask.reshape(mask.shape[-2], mask.shape[-1])
    causal = bool(
        np.array_equal(m2 != 0, np.tril(np.ones((S, S), dtype=bool)))
    )

    if causal not in _prog_cache:
        _prog_cache[causal] = _build_program(causal)
    nc = _prog_cache[causal]

    # host-side shard prep (bf16, transposed)
    xT = {}
    for b in range(B):
        xT[("q", b)] = np.ascontiguousarray(query[b].T).astype(BF16)
        xT[("k", b)] = np.ascontiguousarray(key[b].T).astype(BF16)
        xT[("v", b)] = np.ascontiguousarray(value[b].T).astype(BF16)
    maskT_np = None
    if not causal:
        maskT_np = np.ascontiguousarray((m2 != 0).T).astype(BF16)

    in_maps = []
    for c in range(NCORES):
        b, g = divmod(c, GROUPS)
        rows = slice(HD * g, HD * (g + 1))
        im = {
            "xqT": xT[("q", b)],
            "xkT": xT[("k", b)],
            "xvT": xT[("v", b)],
            "wqT": np.ascontiguousarray(W_q[rows, :].T).astype(BF16),
            "wkT": np.ascontiguousarray(W_k[rows, :].T).astype(BF16),
            "wvT": np.ascontiguousarray(W_v[rows, :].T).astype(BF16),
            "woT": np.ascontiguousarray(W_o[:, rows].T).astype(BF16),
        }
        if not causal:
            im["maskT"] = maskT_np
        in_maps.append(im)

    res = run_bass_kernel_spmd(nc, in_maps, core_ids=list(range(NCORES)))

    out = np.zeros((B, S, D), dtype=np.float32)
    for c in range(NCORES):
        b = c // GROUPS
        out[b] += res.results[c]["yT"].T.astype(np.float32)
    return out
